# revision 1
# baseline (speedup 1.0000x reference)
"""Trainium2 Bass kernel for nn_BaseEncLoss (histogram_binning).

Math: reference loss = mean over (B, nc, H, W) of BCE(sigmoid(preds), se)
where se is the per-grid-cell class-presence map from the downsampled
targets.  Using log_sigmoid(p) - log_sigmoid(-p) = p, the elementwise loss
-(se*logp + (1-se)*log1mp) simplifies to softplus(p) - se*p, so

    loss = (S1 - S2) / numel
    S1   = sum softplus(preds)          (softplus = Ln(Exp(p) + 1) on ACT)
    S2   = sum_cells presence(cell, c) * cellsum(preds over cell)

Per-core work (pure data parallel over the batch): 2 images.

Engine split per core:
  ACT   exp + ln(1+x) (in place, fused row-accumulation for the S1 sums).
  DVE   preds 16-col segment sums, label extraction, per-class bit unpack,
        psum compares/copies, per-tile S2 partials.
  PE    16-row group sums via 0/1 block-selection matmuls.
  GPSIMD  iota, umsk int->f32 convert, output DMA (int32 bitwise ops and
        tensor_scalar are DVE-only on this silicon).
  DMA   preds tiles alternate between the two HWDGE rings (sync/scalar);
        target rows interleave one chunk per preds-tile slot; preds are
        streamed small-tiles-first so ACT starts early, and the targets
        bitmask chain is spread over three tile slots to avoid starving
        the preds segment reductions (which gate tile buffer recycling).

Presence histogram without per-class compare passes: for labels t in
[0, 19), (t + 127) << 23 bitcast to f32 is exactly 2^t; converting back to
int32 gives exactly 1 << t.  A bitwise-or segment reduction then collects a
per-(row, cell-column) class bitmask; per-class presence falls out of tiny
(bm >> k) & 1 unpacks followed by the same selection matmul used for the
preds cell sums.

The activation-table registry handed to Bacc's table-load pass is reduced
to the one set containing both Exp and Ln ('natural_log_exp_and_others')
so the pass emits a single ACT_TABLE_LOAD instead of bouncing between the
exp-only and ln-only sets on every tile (~2.7us per switch on HW).
"""

import sys

sys.path.insert(0, "/opt/trn_rl_repo")

from contextlib import ExitStack

import numpy as np

import concourse.bass as bass
import concourse.tile as tile
from concourse import bacc, mybir
from concourse import bass_utils

N_CORES = 8
FULL_B, CL, H, W = 16, 19, 512, 512
G = 16

F32 = mybir.dt.float32
BF16 = mybir.dt.bfloat16
I32 = mybir.dt.int32
AF = mybir.ActivationFunctionType
OP = mybir.AluOpType
AX = mybir.AxisListType

_COMBINED_SET = "natural_log_exp_and_others"
_tables_patched = False


def _patch_act_tables():
    """Make the act-table-load pass resolve Exp/Ln/Copy to the combined set.

    The pass greedily picks the first table containing each function, which
    alternates exp_and_others / natural_log per tile.  Emptying every other
    set (positions preserved, so act_func_set_id still indexes
    act_info.json correctly for walrus) forces one load of the combined set.
    """
    global _tables_patched
    if _tables_patched:
        return
    from concourse.hw_specs import get_activation_tables as real_gat

    def combined_only(arch):
        tabs = real_gat(arch)
        assert _COMBINED_SET in tabs, sorted(tabs)
        return {
            name: (fns if name == _COMBINED_SET else set())
            for name, fns in tabs.items()
        }

    bacc.get_activation_tables = combined_only
    _tables_patched = True


def build_program(b2, cl, h, w, g, tgt_cols, colstep, n_cores):
    """Build the per-core Bass program.

    b2: images per core; tgt_cols: targets row length in int32 units
    (2*w for int32 targets, 4*w for int64 viewed as int32);
    colstep: int32 stride between consecutive even-column labels.
    """
    _patch_act_tables()
    ch = h // 128          # partition chunks per image plane
    wseg = w // g          # cell columns
    seg = ch * wseg        # free size after 16-col segment reduce
    groups = 128 // g      # partition groups per chunk (8)

    nc = bacc.Bacc(
        "TRN2",
        target_bir_lowering=False,
        debug=False,
        enable_asserts=False,
        num_devices=n_cores,
    )
    preds_t = nc.dram_tensor("preds_sh", (b2, cl, h, w), F32, kind="ExternalInput").ap()
    tgt_t = nc.dram_tensor(
        "targets_sh", (b2, 2 * h, tgt_cols), I32, kind="ExternalInput"
    ).ap()
    out_t = nc.dram_tensor("out_sh", (2, 1), F32, kind="ExternalOutput").ap()

    # preds tile covers `pl` class-planes at once (2 when cl is even-ish)
    plane = ch * w
    n_acc = b2 * (2 + (cl - 2) // 2 + (cl - 2) % 2)

    with tile.TileContext(nc) as tc, ExitStack() as ctx:
        consts = ctx.enter_context(tc.tile_pool(name="consts", bufs=1))
        # sel[p, grp] = 1 iff p // g == grp (iota -> shift -> compare)
        sel = consts.tile([128, groups], F32)
        pidx = consts.tile([128, 1], I32)
        nc.gpsimd.iota(pidx[:], [[0, 1]], base=0, channel_multiplier=1)
        gidx = consts.tile([128, 1], I32)
        nc.vector.tensor_scalar(gidx[:], pidx[:], 4, None, OP.arith_shift_right)
        for grp in range(groups):
            nc.vector.tensor_scalar(
                sel[:, grp : grp + 1], gidx[:], grp, None, OP.is_equal
            )
        ones = consts.tile([128, 1], F32)
        nc.vector.memset(ones[:], 1.0)
        acc1 = consts.tile([128, n_acc], F32)
        acc2 = consts.tile([groups, n_acc], F32)

        pp = ctx.enter_context(tc.tile_pool(name="pp", bufs=5))
        qp = ctx.enter_context(tc.tile_pool(name="qp", bufs=2))
        exp_ = ctx.enter_context(tc.tile_pool(name="exp", bufs=2))
        trp = ctx.enter_context(tc.tile_pool(name="trp", bufs=2))
        pwp = ctx.enter_context(tc.tile_pool(name="pwp", bufs=1))
        orp = ctx.enter_context(tc.tile_pool(name="orp", bufs=1))
        srp = ctx.enter_context(tc.tile_pool(name="srp", bufs=3))
        ump = ctx.enter_context(tc.tile_pool(name="ump", bufs=1))
        big = ctx.enter_context(tc.tile_pool(name="big", bufs=1))
        s2p = ctx.enter_context(tc.tile_pool(name="s2p", bufs=2))
        psc = ctx.enter_context(tc.tile_pool(name="psc", bufs=2, space="PSUM"))
        pss = ctx.enter_context(tc.tile_pool(name="pss", bufs=2, space="PSUM"))
        psf = ctx.enter_context(tc.tile_pool(name="psf", bufs=1, space="PSUM"))
        fin = ctx.enter_context(tc.tile_pool(name="fin", bufs=1))


        # Preds are streamed in a plan of small-first tiles so ACT starts as
        # soon as the first 1MB plane lands; the targets phase is emitted
        # after two tiles so its DMA + DVE bitmask work fills scheduler slack
        # mid-stream; S2 partial products are accumulated per tile to avoid a
        # serial tail.  (Pairing machinery kept but disabled: latency chains
        # cost more than the ACT busy it saves under the static schedule.)
        paired_tis = ()
        plan = [1, 1] + [2] * ((cl - 2) // 2) + [1] * ((cl - 2) % 2)
        ntiles = len(plan)

        for b in range(b2):
            cs = big.tile([groups, cl * seg], F32, tag="cs")
            pw = pwp.tile([128, ch * w], I32, tag="pw")
            pres = None
            bm = None
            umsk = None
            next_stage = 0
            pending_ln = None
            s2_done = 0
            k = 0

            def emit_s2(upto):
                # per-tile S2 partial products (needs pres)
                nonlocal s2_done
                while s2_done < upto:
                    kk, npl = tile_ks[s2_done]
                    pr = s2p.tile([groups, 2 * seg], F32, tag="pr")
                    nc.vector.tensor_mul(
                        pr[:, 0 : npl * seg],
                        pres[:, kk * seg : (kk + npl) * seg],
                        cs[:, kk * seg : (kk + npl) * seg],
                    )
                    nc.vector.tensor_reduce(
                        acc2[:, b * ntiles + s2_done : b * ntiles + s2_done + 1],
                        pr[:, 0 : npl * seg],
                        AX.X,
                        OP.add,
                    )
                    s2_done += 1

            tile_ks = []
            for ti, pl in enumerate(plan):
                fsz = pl * plane
                tile_ks.append((k, pl))
                pt = pp.tile([128, 2 * plane], F32, tag="pt")
                src = preds_t[b, k : k + pl].rearrange("q (c p) x -> p q c x", p=128)
                eng = nc.sync if (ti % 2 == 0) else nc.scalar
                eng.dma_start(
                    pt[:, 0:fsz].rearrange("p (q c x) -> p q c x", q=pl, x=w), src
                )
                ex = exp_.tile([128, 2 * plane], F32, tag="ex")
                nc.scalar.activation(ex[:, 0:fsz], pt[:, 0:fsz], AF.Exp)
                a_i = b * ntiles + ti
                if pl == 2 and ti in paired_tis:
                    ea = ex[:, 0:plane]
                    eb = ex[:, plane : 2 * plane]
                    q = qp.tile([128, plane], F32, tag="q")
                    nc.vector.scalar_tensor_tensor(
                        q[:], ea, 1.0, eb, OP.add, OP.mult
                    )
                    nc.vector.tensor_tensor(q[:], q[:], ea, OP.add)
                    ln_in, ln_sz = q[:], plane
                else:
                    ln_in, ln_sz = ex[:], fsz

                def emit_ln(ln_in=ln_in, ln_sz=ln_sz, a_i=a_i):
                    # ln(1 + x) written in place over its input
                    nc.scalar.activation(
                        ln_in[:, 0:ln_sz],
                        ln_in[:, 0:ln_sz],
                        AF.Ln,
                        bias=1.0,
                        accum_out=acc1[:, a_i : a_i + 1],
                    )

                if pending_ln is not None:
                    pending_ln()
                pending_ln = emit_ln

                sg2 = srp.tile([128, 2 * seg], F32, tag="seg")
                nc.vector.tensor_reduce(
                    sg2[:, 0 : pl * seg],
                    pt[:, 0:fsz].rearrange("p (e x s) -> p (e x) s", s=g, e=pl * ch),
                    AX.X,
                    OP.add,
                )
                csp = pss.tile([groups, 2 * seg], F32, tag="csp")
                for j in range(pl):
                    nc.tensor.matmul(
                        csp[:, bass.ts(j, seg)],
                        sel[:],
                        sg2[:, bass.ts(j, seg)],
                        start=True,
                        stop=True,
                    )
                nc.vector.tensor_copy(
                    cs[:, k * seg : (k + pl) * seg], csp[:, 0 : pl * seg]
                )
                k += pl

                if ti < ch:
                    # ---- targets, spread out: one even-row chunk DMA +
                    # fused downsample/exponent-field extraction per preds
                    # tile slot, so the raws interleave with preds tiles on
                    # the sync ring instead of lumping.
                    c = ti
                    raw = trp.tile([128, tgt_cols], I32, tag="raw")
                    tsrc = (
                        tgt_t[b]
                        .rearrange("(r two) x -> two r x", two=2)[0]
                        .rearrange("(c p) x -> c p x", p=128)[c]
                    )
                    nc.sync.dma_start(raw[:], tsrc)
                    ext = raw[:].rearrange("p (x s) -> p x s", s=colstep)[:, :, 0]
                    # (t + 127) * 2^23 == f32 bit pattern of 2^t; all-arith
                    # op pair (walrus rejects mixed bitwise/arith), exact in
                    # both int32 and f32 ALU typings.
                    nc.vector.tensor_scalar(
                        pw[:, bass.ts(c, w)], ext, 127.0, float(1 << 23),
                        OP.add, OP.mult,
                    )
                def stage1():
                    # bitmask chain part 1: 1<<t (in-place convert of the
                    # exponent-field patterns) and the or-tree
                    nonlocal bm
                    nc.vector.tensor_copy(pw[:], pw[:].bitcast(F32))  # 1 << t
                    cur = pw
                    width = g
                    while width > 1:
                        width //= 2
                        nxt = orp.tile([128, seg * width], I32, tag=f"or{width}")
                        a = cur[:].rearrange("p (e s) -> p e s", s=2 * width)
                        nc.vector.tensor_tensor(
                            nxt[:].rearrange("p (e s) -> p e s", s=width),
                            a[:, :, 0:width],
                            a[:, :, width : 2 * width],
                            OP.bitwise_or,
                        )
                        cur = nxt
                    bm = cur

                def stage2():
                    # part 2: unpack per class (bitwise cannot cast: int bits
                    # into the f32 tile via a bitcast view, convert in place)
                    nonlocal umsk
                    umsk = ump.tile([128, cl * seg], F32, tag="umsk")
                    umski = umsk[:].bitcast(I32)
                    for kq in range(cl):
                        nc.vector.tensor_scalar(
                            umski[:, bass.ts(kq, seg)], bm[:], kq, 1,
                            OP.logical_shift_right, OP.bitwise_and,
                        )
                    nc.gpsimd.tensor_copy(umsk[:], umski)

                def stage3():
                    # part 3: row-group counts and presence
                    nonlocal pres
                    pres = big.tile([groups, cl * seg], F32, tag="pres")
                    kgrp = 4
                    for k0 in range(0, cl, kgrp):
                        kn = min(kgrp, cl - k0)
                        cps = psc.tile([groups, kgrp * seg], F32, tag="cps")
                        for j in range(kn):
                            nc.tensor.matmul(
                                cps[:, bass.ts(j, seg)],
                                sel[:],
                                umsk[:, bass.ts(k0 + j, seg)],
                                start=True,
                                stop=True,
                            )
                        nc.vector.tensor_scalar(
                            pres[:, k0 * seg : (k0 + kn) * seg],
                            cps[:, 0 : kn * seg],
                            0.5,
                            None,
                            OP.is_ge,
                        )

                stages = (stage1, stage2, stage3)
                while next_stage < len(stages) and ti == ch + next_stage:
                    stages[next_stage]()
                    next_stage += 1
                if ti > ch + 2:
                    emit_s2(ti - ch - 2)

            while next_stage < len(stages):
                stages[next_stage]()
                next_stage += 1
            if pending_ln is not None:
                pending_ln()
            emit_s2(ntiles)

        # ---- final: (S1, S2) partials -> out_sh[2, 1].
        final = fin.tile([128, 2], F32)
        nc.vector.memset(final[:], 0.0)
        nc.vector.tensor_reduce(final[:, 0:1], acc1[:], AX.X, OP.add)
        nc.vector.tensor_reduce(final[0:groups, 1:2], acc2[:], AX.X, OP.add)
        fp = psf.tile([2, 1], F32, tag="fp")
        nc.tensor.matmul(fp[:], final[:], ones[:], start=True, stop=True)
        osb = fin.tile([2, 1], F32)
        nc.vector.tensor_copy(osb[:], fp[:])
        nc.gpsimd.dma_start(out_t, osb[:])

    nc.compile()
    return nc


_CACHE: dict = {}


def kernel(preds: np.ndarray, targets: np.ndarray, grid_size=16) -> np.ndarray:
    preds = np.asarray(preds)
    targets = np.asarray(targets)
    assert preds.shape == (FULL_B, CL, H, W) and preds.dtype == np.float32
    assert targets.shape == (FULL_B, 2 * H, 2 * W)
    assert int(np.asarray(grid_size)) == G

    if targets.dtype == np.int64:
        if not targets.flags.c_contiguous:
            targets = np.ascontiguousarray(targets)
        tgt_i32 = targets.view(np.int32).reshape(FULL_B, 2 * H, 4 * W)
        colstep = 4
    elif targets.dtype == np.int32:
        tgt_i32 = targets
        colstep = 2
    else:
        raise ValueError(f"unsupported targets dtype {targets.dtype}")

    b2 = FULL_B // N_CORES
    key = (b2, targets.dtype.str)
    if key not in _CACHE:
        _CACHE[key] = build_program(
            b2, CL, H, W, G, tgt_i32.shape[2], colstep, N_CORES
        )
    nc = _CACHE[key]

    in_maps = [
        {
            "preds_sh": preds[i * b2 : (i + 1) * b2],
            "targets_sh": tgt_i32[i * b2 : (i + 1) * b2],
        }
        for i in range(N_CORES)
    ]
    res = bass_utils.run_bass_kernel_spmd(nc, in_maps, core_ids=list(range(N_CORES)))
    global LAST_RESULTS
    LAST_RESULTS = res

    s1 = 0.0
    s2 = 0.0
    for r in res.results:
        out = r["out_sh"]
        s1 += float(out[0, 0])
        s2 += float(out[1, 0])
    numel = preds.size
    return np.asarray((s1 - s2) / numel, dtype=np.float32)


LAST_RESULTS = None

